# revision 47
# baseline (speedup 1.0000x reference)
"""Trainium2 Bass kernel for nn_KKLayer (spectral channel-mix layer).

Math identity: the reference computes
    y = Re(IFFT2((A + iB) . conj(FFT2(x))))            (channel mix in freq domain)
Since channel mixing commutes with the spatial FFT and, for real x,
IFFT2(conj(FFT2(x))) is x spatially "negated" (h -> (-h) mod H, w -> (-w) mod W),
the whole layer collapses to
    y[b,o,h,w] = sum_i A[o,i] * x[b,i,(H-h)%H,(W-w)%W]
(betas drop out of the real part entirely).

The (h,w) flip is folded into the host-side shard step (a fancy-index while
casting x to bf16), so the device kernel is a pure streaming channel-matmul:

  per core (data-parallel over batch, 8 batches -> 8 cores):
    - load alphas^T (stationary weights, bf16) + pre-flipped x[b] (bf16),
      input transfers alternating across both HWDGE rings (sync + scalar)
    - per 1024-col chunk: 2 bf16 matmuls [K=128,M=128,N=512] -> PSUM fp32
      (4 rotating 2-bank slots), PSUM->SBUF downconvert copies pair-wise
      on one engine (engines alternate per pair), 512KB bf16 DMA out per
      pair
    - host upcasts bf16 -> fp32

bf16 I/O halves HBM traffic (8.4MB/core) and runs the PE at 1 cycle/row
instead of fp32's 4 (rel err ~4e-3 << 2e-2 gate).
"""

import ml_dtypes
import numpy as np

import concourse.bass as bass
import concourse.bacc as bacc
import concourse.mybir as mybir
from concourse import tile
from concourse.bass_utils import run_bass_kernel_spmd

B, CIN, COUT, H, W = 8, 128, 128, 128, 128
HW = H * W          # 16384
BLK = 512           # matmul free dim (one PSUM bank of fp32)
# input DMAs: symmetric transfers alternating between the two HWDGE
# rings.  Both rings drain at the same pace, so arrival order matches
# compute order, while the ring round-robin keeps aggregate rate at
# ~430GB/s (a single FIFO ring caps at ~300 due to boundary bubbles).
# Smaller first transfers prime the compute pipeline sooner.  Note there
# are only 8 DMAHW completion lanes: the 9th+ DMA stalls at issue until
# an earlier transfer's completion receipt frees a lane (~1.4us if the
# receipt is still in flight) -- one reason not to split finer than this
# (uniform [2048]*8 measures within noise of this config).
IN_COLS = [1024] * 4 + [2048] * 6
# compute/copy/out granularity (cols): one PSUM allocation (4 banks, 2
# slots = same 4096-col pipeline runway as 4x2-bank), ONE [128,2048]
# PSUM->SBUF copy per chunk on an alternating engine (a single engine
# per y tile -- cross-engine writes to one tile serialize via
# semaphores), one 512KB out-DMA per chunk.  Minimal instruction/sem
# count: on this power-capped part total activity feeds throttling.
CHUNK = 2048
N_CORES = 8

F32 = mybir.dt.float32
BF16 = mybir.dt.bfloat16
NP_BF16 = ml_dtypes.bfloat16

# (-h) % H index for the host-side spatial flip
_FLIP = (-np.arange(H)) % H


def _build_nc():
    nc = bacc.Bacc(None, target_bir_lowering=False)
    x = nc.dram_tensor("x", [CIN, HW], BF16, kind="ExternalInput")
    wT = nc.dram_tensor("wT", [CIN, COUT], BF16, kind="ExternalInput")
    y = nc.dram_tensor("y", [COUT, HW], BF16, kind="ExternalOutput")

    in_offs = np.cumsum([0] + IN_COLS)
    with tile.TileContext(nc) as tc:
        with (
            tc.tile_pool(name="wp", bufs=1) as wpool,
            tc.tile_pool(name="xp", bufs=1) as xpool,
            tc.tile_pool(name="yp", bufs=1) as ypool,
            tc.tile_pool(name="ps", bufs=2, space="PSUM") as pspool,
        ):
            # all input DMAs up front, split across BOTH HWDGE rings (SP via
            # nc.sync, ACT via nc.scalar): two transfers are always in
            # flight, so the SDMA engines' round-robin covers the per-
            # transfer boundary bubbles that cap a single FIFO ring at
            # ~300GB/s.  Compute is bus-hidden and order-independent.
            # w rides the sync ring (32KB, ~0.15us) so both rings carry an
            # equal share of x and odd chunks arrive on schedule
            w_t = wpool.tile([CIN, COUT], BF16)
            nc.sync.dma_start(w_t[:], wT[:])
            xin = []
            for k, cols in enumerate(IN_COLS):
                t = xpool.tile([CIN, cols], BF16, tag=f"x{k}", name=f"xch{k}")
                eng = nc.sync if k % 2 == 0 else nc.scalar
                eng.dma_start(t[:], x[:, in_offs[k]: in_offs[k + 1]])
                xin.append(t)

            # (A PE p-state warm-up was tried here and reverted: junk
            # matmuls never ramp past ~634ns/matmul and the power governor
            # responds with more throttle_active, canceling the gain.)
            for c in range(HW // CHUNK):
                base = c * CHUNK
                ps = pspool.tile([COUT, CHUNK], F32, tag="ps", name=f"ps{c}")
                for j in range(CHUNK // BLK):
                    # which input tile holds this 512-col block (a chunk may
                    # span two of the tapered input transfers)
                    b0 = base + BLK * j
                    k = int(np.searchsorted(in_offs, b0, side="right")) - 1
                    lo = b0 - in_offs[k]
                    nc.tensor.matmul(
                        ps[:, BLK * j: BLK * (j + 1)],
                        w_t[:],
                        xin[k][:, lo: lo + BLK],
                        start=True,
                        stop=True,
                    )
                yt = ypool.tile([COUT, CHUNK], BF16, tag=f"y{c}", name=f"ych{c}")
                if c % 2 == 0:
                    nc.vector.tensor_copy(yt[:], ps[:])
                else:
                    nc.scalar.copy(yt[:], ps[:])
                nc.sync.dma_start(y[:, base: base + CHUNK], yt[:])
    nc.compile()
    return nc


_NC_CACHE = {}


def _get_nc():
    if "nc" not in _NC_CACHE:
        _NC_CACHE["nc"] = _build_nc()
    return _NC_CACHE["nc"]


def make_in_maps(x, alphas):
    """Per-core input maps: bf16, with the (h,w) flip pre-applied to x."""
    x16 = np.asarray(x, dtype=np.float32).astype(NP_BF16)
    wT = np.ascontiguousarray(
        np.asarray(alphas, dtype=np.float32).T
    ).astype(NP_BF16)
    maps = []
    for c in range(N_CORES):
        xf = x16[c][:, _FLIP][:, :, _FLIP]
        maps.append(
            {"x": np.ascontiguousarray(xf.reshape(CIN, HW)), "wT": wT}
        )
    return maps


def kernel(x, alphas, betas=None, **_unused):
    nc = _get_nc()
    in_maps = make_in_maps(x, alphas)
    res = run_bass_kernel_spmd(nc, in_maps, core_ids=list(range(N_CORES)))
    out = np.stack(
        [
            res.results[c]["y"].astype(np.float32).reshape(COUT, H, W)
            for c in range(N_CORES)
        ]
    )
    return out


# revision 50
# speedup vs baseline: 1.2157x; 1.2157x over previous
"""Trainium2 Bass kernel for nn_KKLayer (spectral channel-mix layer).

Math identity: the reference computes
    y = Re(IFFT2((A + iB) . conj(FFT2(x))))            (channel mix in freq domain)
Since channel mixing commutes with the spatial FFT and, for real x,
IFFT2(conj(FFT2(x))) is x spatially "negated" (h -> (-h) mod H, w -> (-w) mod W),
the whole layer collapses to
    y[b,o,h,w] = sum_i A[o,i] * x[b,i,(H-h)%H,(W-w)%W]
(betas drop out of the real part entirely).

The (h,w) flip is folded into the host-side shard step (a fancy-index while
casting x to bf16), so the device kernel is a pure streaming channel-matmul:

  per core (data-parallel over batch, 8 batches -> 8 cores):
    - load alphas^T (stationary weights, bf16) + pre-flipped x[b] (bf16),
      input transfers alternating across both HWDGE rings (sync + scalar)
    - per 1024-col chunk: 2 bf16 matmuls [K=128,M=128,N=512] -> PSUM fp32
      (4 rotating 2-bank slots), PSUM->SBUF downconvert copies pair-wise
      on one engine (engines alternate per pair), 512KB bf16 DMA out per
      pair
    - host upcasts bf16 -> fp32

bf16 I/O halves HBM traffic (8.4MB/core) and runs the PE at 1 cycle/row
instead of fp32's 4 (rel err ~4e-3 << 2e-2 gate).
"""

import ml_dtypes
import numpy as np

import concourse.bass as bass
import concourse.bacc as bacc
import concourse.mybir as mybir
from concourse import tile
from concourse.bass_utils import run_bass_kernel_spmd

B, CIN, COUT, H, W = 8, 128, 128, 128, 128
HW = H * W          # 16384
BLK = 512           # matmul free dim (one PSUM bank of fp32)
# input DMAs: symmetric transfers alternating between the two HWDGE
# rings.  Both rings drain at the same pace, so arrival order matches
# compute order, while the ring round-robin keeps aggregate rate at
# ~430GB/s (a single FIFO ring caps at ~300 due to boundary bubbles).
# Smaller first transfers prime the compute pipeline sooner.  Note there
# are only 8 DMAHW completion lanes: the 9th+ DMA stalls at issue until
# an earlier transfer's completion receipt frees a lane (~1.4us if the
# receipt is still in flight) -- one reason not to split finer than this
# (uniform [2048]*8 measures within noise of this config).
IN_COLS = [1024] * 4 + [2048] * 6
# compute/copy granularity (cols): one PSUM allocation (2 banks), 4 slots
# deep -- the 4-slot slack is REQUIRED by the dual-ring input (chunks
# arrive in ring pairs; a 2-slot coarse pipeline cannot absorb the
# jitter and measures ~8us slower).  Copies for a PAIR of chunks run
# back-to-back on ONE engine into one shared y tile (same-engine tile
# writes are ordered for free; cross-engine writes to one tile serialize
# via semaphores), engines alternate per pair, one 512KB out per pair.
CHUNK = 1024
PAIR = 2 * CHUNK
N_CORES = 8

F32 = mybir.dt.float32
BF16 = mybir.dt.bfloat16
NP_BF16 = ml_dtypes.bfloat16

# (-h) % H index for the host-side spatial flip
_FLIP = (-np.arange(H)) % H


def _build_nc():
    nc = bacc.Bacc(None, target_bir_lowering=False)
    x = nc.dram_tensor("x", [CIN, HW], BF16, kind="ExternalInput")
    wT = nc.dram_tensor("wT", [CIN, COUT], BF16, kind="ExternalInput")
    y = nc.dram_tensor("y", [COUT, HW], BF16, kind="ExternalOutput")

    in_offs = np.cumsum([0] + IN_COLS)
    with tile.TileContext(nc) as tc:
        with (
            tc.tile_pool(name="wp", bufs=1) as wpool,
            tc.tile_pool(name="xp", bufs=1) as xpool,
            tc.tile_pool(name="yp", bufs=1) as ypool,
            tc.tile_pool(name="ps", bufs=4, space="PSUM") as pspool,
        ):
            # all input DMAs up front, split across BOTH HWDGE rings (SP via
            # nc.sync, ACT via nc.scalar): two transfers are always in
            # flight, so the SDMA engines' round-robin covers the per-
            # transfer boundary bubbles that cap a single FIFO ring at
            # ~300GB/s.  Compute is bus-hidden and order-independent.
            # w rides the sync ring (32KB, ~0.15us) so both rings carry an
            # equal share of x and odd chunks arrive on schedule
            w_t = wpool.tile([CIN, COUT], BF16)
            nc.sync.dma_start(w_t[:], wT[:])
            xin = []
            for k, cols in enumerate(IN_COLS):
                t = xpool.tile([CIN, cols], BF16, tag=f"x{k}", name=f"xch{k}")
                eng = nc.sync if k % 2 == 0 else nc.scalar
                eng.dma_start(t[:], x[:, in_offs[k]: in_offs[k + 1]])
                xin.append(t)

            # (A PE p-state warm-up was tried here and reverted: junk
            # matmuls never ramp past ~634ns/matmul and the power governor
            # responds with more throttle_active, canceling the gain.)
            for c in range(HW // CHUNK):
                base = c * CHUNK
                ps = pspool.tile([COUT, CHUNK], F32, tag="ps", name=f"ps{c}")
                for j in range(CHUNK // BLK):
                    # which input tile holds this 512-col block (a chunk may
                    # span two of the tapered input transfers)
                    b0 = base + BLK * j
                    k = int(np.searchsorted(in_offs, b0, side="right")) - 1
                    lo = b0 - in_offs[k]
                    nc.tensor.matmul(
                        ps[:, BLK * j: BLK * (j + 1)],
                        w_t[:],
                        xin[k][:, lo: lo + BLK],
                        start=True,
                        stop=True,
                    )
                t = c // 2
                half = (c % 2) * CHUNK
                if half == 0:
                    yt = ypool.tile(
                        [COUT, PAIR], BF16, tag=f"y{t}", name=f"ych{t}"
                    )
                    yts = yt
                else:
                    yt = yts
                if t % 2 == 0:
                    nc.vector.tensor_copy(yt[:, half: half + CHUNK], ps[:])
                else:
                    nc.scalar.copy(yt[:, half: half + CHUNK], ps[:])
                if half:
                    nc.sync.dma_start(y[:, base - CHUNK: base + CHUNK], yt[:])
    nc.compile()
    return nc


_NC_CACHE = {}


def _get_nc():
    if "nc" not in _NC_CACHE:
        _NC_CACHE["nc"] = _build_nc()
    return _NC_CACHE["nc"]


def make_in_maps(x, alphas):
    """Per-core input maps: bf16, with the (h,w) flip pre-applied to x."""
    x16 = np.asarray(x, dtype=np.float32).astype(NP_BF16)
    wT = np.ascontiguousarray(
        np.asarray(alphas, dtype=np.float32).T
    ).astype(NP_BF16)
    maps = []
    for c in range(N_CORES):
        xf = x16[c][:, _FLIP][:, :, _FLIP]
        maps.append(
            {"x": np.ascontiguousarray(xf.reshape(CIN, HW)), "wT": wT}
        )
    return maps


def kernel(x, alphas, betas=None, **_unused):
    nc = _get_nc()
    in_maps = make_in_maps(x, alphas)
    res = run_bass_kernel_spmd(nc, in_maps, core_ids=list(range(N_CORES)))
    out = np.stack(
        [
            res.results[c]["y"].astype(np.float32).reshape(COUT, H, W)
            for c in range(N_CORES)
        ]
    )
    return out
